# revision 12
# baseline (speedup 1.0000x reference)
"""WaveNet (NvWaveNet) forward kernel for 8 Trainium2 NeuronCores.

Sharding: 8 cores = 4 batches x 2 time-halves, uniform SPMD program.
Each core computes a width-9216 column window through the whole net:
  half 0: seq cols [0, 9216)      -> owns [0, 9216)
  half 1: seq cols [7168, 16384)  -> owns [9216, 16384)
           (first 2048 cols are causal-halo recompute; WaveNet receptive
            decay makes the 2048-halo truncation error ~3.5e-5 rel.)

Depth-first schedule: groups of THREE 512-col tiles run through all 20
layers together (3 independent dependency chains hide the per-layer
latency); skip contributions accumulate in PSUM across the whole stack
(3 tiles x 2 banks = 6 of 8 banks); final 1x1 convs consume skip PSUM
directly via their relu.

Gate (exact): tanh(a)*sigmoid(b) == 2*(sigmoid(2a)-0.5)*sigmoid(b).
Two sigmoids write A and B into separate base-0 SBUF tiles (two-input
DVE ops require equal SBUF base partitions), then one fused
scalar_tensor_tensor computes z' = (A-0.5)*B = z/2; the 2x is folded
into W_skip/W_out host-side. The conditioning matmul is emitted FIRST
in each PSUM accumulation so only the 3 dilated taps sit on the
layer-to-layer critical path.

Math folding (host-side):
  g-space residual: g_l = g_{l-1} + (W_out[l] * s^-l) z_l   (s = sqrt(1/2))
  h_{l-1} = s^l g_{l-1} + r_{l-1}; r folded into per-layer gate biases.
  skip scalings folded into W_skip so skips accumulate as a plain sum.
Per-layer weights are SBUF-resident in bf16 (stationary operand only;
moving operands stay float32r, so matmuls run at full PE rate).
"""

import sys
import numpy as np

sys.path.insert(0, "/opt/trn_rl_repo")

LAYERS = 20
KW = 3
OUT_CH = 256
RES_CH = 64
GATE_CH = 128
SKIP_CH = 256
CIN_CH = 80
T = 16384
B = 4
S = 0.7071067811865476

W = 9216           # per-core compute width
TILE = 512
NS = 3             # tiles per group (PSUM-limited)
NT = W // TILE     # 18
H1_START = T - W   # 7168: seq col where half-1 cores start computing
DILS = [2 ** (l % 10) for l in range(LAYERS)]

_CACHE = {}


def _build_nc():
    from contextlib import ExitStack
    import concourse.bass as bass
    import concourse.tile as tile
    from concourse import bacc, mybir

    f32 = mybir.dt.float32
    f32r = mybir.dt.float32r
    bf16 = mybir.dt.bfloat16
    AF = mybir.ActivationFunctionType
    ALU = mybir.AluOpType

    nc = bacc.Bacc()

    # ---- DRAM parameters ----
    x_d = nc.declare_dram_parameter("x", [OUT_CH, W + 1], bf16, isOutput=False)
    c_d = nc.declare_dram_parameter("c", [CIN_CH, W], bf16, isOutput=False)
    wdil_d = nc.declare_dram_parameter("wdil", [LAYERS, RES_CH, 3 * GATE_CH], f32r, isOutput=False)
    wc_d = nc.declare_dram_parameter("wc", [LAYERS, CIN_CH, GATE_CH], bf16, isOutput=False)
    wskip_d = nc.declare_dram_parameter("wskip", [LAYERS, RES_CH, SKIP_CH], bf16, isOutput=False)
    wout_d = nc.declare_dram_parameter("wout", [LAYERS, RES_CH, RES_CH], bf16, isOutput=False)
    wfirst_d = nc.declare_dram_parameter("wfirst", [2, 128, 2 * RES_CH], bf16, isOutput=False)
    wlast1_d = nc.declare_dram_parameter("wlast1", [2, 128, SKIP_CH], bf16, isOutput=False)
    wlast2_d = nc.declare_dram_parameter("wlast2", [2, 128, OUT_CH], bf16, isOutput=False)
    biases_d = nc.declare_dram_parameter("biases", [128, 32], f32, isOutput=False)
    out_d = nc.declare_dram_parameter("out", [OUT_CH, W], f32, isOutput=True)

    with ExitStack() as ctx:
        tc = ctx.enter_context(tile.TileContext(nc))

        # ---- resident constants & weights ----
        cpool = ctx.enter_context(tc.tile_pool(name="consts", bufs=1))
        biases = cpool.tile([128, 32], f32, tag="biases")
        nc.sync.dma_start(biases[:], biases_d[:])
        wfirst = cpool.tile([128, 2, 2 * RES_CH], bf16, tag="wfirst")
        nc.sync.dma_start(wfirst[:], wfirst_d.rearrange("a b c -> b a c"))
        wlast1 = cpool.tile([128, 2, SKIP_CH], bf16, tag="wlast1")
        nc.sync.dma_start(wlast1[:], wlast1_d.rearrange("a b c -> b a c"))
        wlast2 = cpool.tile([128, 2, OUT_CH], bf16, tag="wlast2")
        nc.sync.dma_start(wlast2[:], wlast2_d.rearrange("a b c -> b a c"))

        # all per-layer weights resident in bf16: one bulk DMA per family
        wdil = cpool.tile([RES_CH, LAYERS, 3 * GATE_CH], f32r, tag="wdil")
        nc.sync.dma_start(wdil[:], wdil_d.rearrange("l r c -> r l c"))
        wcl = cpool.tile([CIN_CH, LAYERS, GATE_CH], bf16, tag="wc")
        nc.sync.dma_start(wcl[:], wc_d.rearrange("l r c -> r l c"))
        wskip = cpool.tile([RES_CH, LAYERS, SKIP_CH], bf16, tag="wskip")
        nc.sync.dma_start(wskip[:], wskip_d.rearrange("l r c -> r l c"))
        wout = cpool.tile([RES_CH, LAYERS, RES_CH], bf16, tag="wout")
        nc.sync.dma_start(wout[:], wout_d.rearrange("l r c -> r l c"))

        def bias_col(i, p0, p1):
            return biases[p0:p1, i:i + 1]

        # ---- per-layer history buffers: hist[l] holds input of layer l ----
        spool = ctx.enter_context(tc.tile_pool(name="state", bufs=1))
        hist = []
        for l in range(LAYERS):
            wl = 2 * DILS[l] + NS * TILE
            hb = spool.tile([RES_CH, wl], f32r, tag=f"h{l}", name=f"h{l}")
            # only the left-context tail needs zeroing; the rest is
            # written before it is read
            eng = nc.vector if l % 2 == 0 else nc.gpsimd
            eng.memset(hb[:, 0:2 * DILS[l]].bitcast(f32), 0.0)
            hist.append(hb)

        # ---- working pools ----
        xpool = ctx.enter_context(tc.tile_pool(name="xload", bufs=1))
        clpool = ctx.enter_context(tc.tile_pool(name="cload", bufs=NS))
        ztpool = ctx.enter_context(tc.tile_pool(name="zt", bufs=NS))
        fwork = ctx.enter_context(tc.tile_pool(name="fwork", bufs=1))
        opool = ctx.enter_context(tc.tile_pool(name="oout", bufs=1))

        ypool = ctx.enter_context(tc.tile_pool(name="psum_y", bufs=2, space="PSUM"))
        skpool = ctx.enter_context(tc.tile_pool(name="psum_sk", bufs=NS, space="PSUM"))

        # ---- final 1x1 convs for one tile (reads skip PSUM directly) ----
        def emit_final(t, sa, sb):
            t0 = t * TILE
            rs_a = fwork.tile([128, TILE], bf16, tag="fa", name="rs_a")
            nc.scalar.activation(rs_a[:], sa[:], AF.Relu, bias=bias_col(21, 0, 128))
            rs_b = fwork.tile([128, TILE], bf16, tag="fb", name="rs_b")
            nc.scalar.activation(rs_b[:], sb[:], AF.Relu, bias=bias_col(22, 0, 128))

            pp = ypool.tile([128, TILE], f32, tag="y", name="pp")
            nc.tensor.matmul(pp[:], wlast1[:, 0, 0:128], rs_a[:], start=True, stop=False)
            nc.tensor.matmul(pp[:], wlast1[:, 1, 0:128], rs_b[:], start=False, stop=True)
            pq = ypool.tile([128, TILE], f32, tag="y", name="pq")
            nc.tensor.matmul(pq[:], wlast1[:, 0, 128:256], rs_a[:], start=True, stop=False)
            nc.tensor.matmul(pq[:], wlast1[:, 1, 128:256], rs_b[:], start=False, stop=True)

            r1_a = fwork.tile([128, TILE], bf16, tag="fa", name="r1_a")
            nc.scalar.activation(r1_a[:], pp[:], AF.Relu, bias=bias_col(23, 0, 128))
            r1_b = fwork.tile([128, TILE], bf16, tag="fb", name="r1_b")
            nc.scalar.activation(r1_b[:], pq[:], AF.Relu, bias=bias_col(24, 0, 128))

            pu = ypool.tile([128, TILE], f32, tag="y", name="pu")
            nc.tensor.matmul(pu[:], wlast2[:, 0, 0:128], r1_a[:], start=True, stop=False)
            nc.tensor.matmul(pu[:], wlast2[:, 1, 0:128], r1_b[:], start=False, stop=True)
            pv = ypool.tile([128, TILE], f32, tag="y", name="pv")
            nc.tensor.matmul(pv[:], wlast2[:, 0, 128:256], r1_a[:], start=True, stop=False)
            nc.tensor.matmul(pv[:], wlast2[:, 1, 128:256], r1_b[:], start=False, stop=True)

            oa = opool.tile([128, TILE], f32, tag="oa")
            nc.vector.tensor_scalar_add(oa[:], pu[:], bias_col(25, 0, 128))
            ob = opool.tile([128, TILE], f32, tag="ob")
            nc.vector.tensor_scalar_add(ob[:], pv[:], bias_col(26, 0, 128))
            nc.sync.dma_start(out_d[0:128, t0:t0 + TILE], oa[:])
            nc.sync.dma_start(out_d[128:256, t0:t0 + TILE], ob[:])

        def emit_preamble(group):
            """x/c loads, first conv, tanh -> hist0, skip-accumulator allocs."""
            cts, sas, sbs = {}, {}, {}
            for pi, t in enumerate(group):
                t0 = t * TILE
                ct = clpool.tile([CIN_CH, TILE], bf16, tag="ct")
                nc.sync.dma_start(ct[:], c_d[:, t0:t0 + TILE])
                cts[t] = ct
                # first conv (causal k=2): hist0 <- tanh(W_first * x + b_first)
                xa = xpool.tile([128, TILE + 1], bf16, tag="xa")
                xb = xpool.tile([128, TILE + 1], bf16, tag="xb")
                nc.sync.dma_start(xa[:], x_d[0:128, t0:t0 + TILE + 1])
                nc.sync.dma_start(xb[:], x_d[128:256, t0:t0 + TILE + 1])
                pf = ypool.tile([GATE_CH, TILE], f32, tag="y", name=f"pf{t}")
                first = True
                for ci, xt in ((0, xa), (1, xb)):
                    for k in (0, 1):
                        nc.tensor.matmul(
                            pf[0:RES_CH, :], wfirst[:, ci, k * RES_CH:(k + 1) * RES_CH],
                            xt[:, k:k + TILE],
                            start=first, stop=(ci == 1 and k == 1))
                        first = False
                o0 = 2 + pi * TILE
                nc.scalar.activation(hist[0][:, o0:o0 + TILE], pf[0:RES_CH, :],
                                     AF.Tanh, bias=bias_col(20, 0, RES_CH))
                sas[t] = skpool.tile([128, TILE], f32, tag="sa", name=f"sa{t}")
                sbs[t] = skpool.tile([128, TILE], f32, tag="sb", name=f"sb{t}")
            return cts, sas, sbs

        def emit_y(group, l, cts):
            """cond + dilated taps for layer l of all tiles (cond first: only
            the taps gate on the previous layer's residual add)."""
            d = DILS[l]
            hb = hist[l]
            ys = {}
            for pi, t in enumerate(group):
                off = pi * TILE
                y = ypool.tile([GATE_CH, TILE], f32, tag="y", name=f"y{t}")
                nc.tensor.matmul(y[:], wcl[:, l, :], cts[t][:], start=True, stop=False)
                for k in range(3):
                    nc.tensor.matmul(
                        y[:], wdil[:, l, k * GATE_CH:(k + 1) * GATE_CH],
                        hb[:, off + k * d:off + k * d + TILE],
                        start=False, stop=(k == 2))
                ys[t] = y
            return ys

        def emit_sig(t, l, y):
            """two sigmoids -> A, B in separate base-0 SBUF tiles"""
            za = ztpool.tile([RES_CH, TILE], bf16, tag="za", name=f"za{t}", bufs=4)
            nc.scalar.activation(za[:], y[0:RES_CH, :], AF.Sigmoid,
                                 bias=bias_col(l, 0, RES_CH))
            zb = ztpool.tile([RES_CH, TILE], bf16, tag="zb", name=f"zb{t}", bufs=4)
            nc.scalar.activation(zb[:], y[RES_CH:GATE_CH, :], AF.Sigmoid,
                                 bias=bias_col(l, RES_CH, GATE_CH))
            return za, zb

        def emit_y1(t, pi, l, ct):
            """cond + dilated taps for layer l of one tile (cond first: only
            the taps gate on the previous layer's residual add)."""
            d = DILS[l]
            hb = hist[l]
            off = pi * TILE
            y = ypool.tile([GATE_CH, TILE], f32, tag="y", name=f"y{t}")
            nc.tensor.matmul(y[:], wcl[:, l, :], ct[:], start=True, stop=False)
            for k in range(3):
                nc.tensor.matmul(
                    y[:], wdil[:, l, k * GATE_CH:(k + 1) * GATE_CH],
                    hb[:, off + k * d:off + k * d + TILE],
                    start=False, stop=(k == 2))
            return y

        # ---- software-pipelined emission: each tile's block in row l also
        # emits the y-matmuls AND the sigmoids of row l+1, so the y PSUM
        # buffer is consumed within the same row (2-buffer rotation stays
        # fluid); the previous group's finals are emitted after the next
        # group's preamble + row-0 ys/sigmoids ----
        pending_final = None
        for tg in range(0, NT, NS):
            group = tuple(tg + i for i in range(NS))
            cts, sas, sbs = emit_preamble(group)
            zas, zbs = {}, {}
            for pi, t in enumerate(group):
                y = emit_y1(t, pi, 0, cts[t])
                zas[t], zbs[t] = emit_sig(t, 0, y)
            if pending_final is not None:
                for t, sa, sb in pending_final:
                    emit_final(t, sa, sb)
                pending_final = None

            for l in range(LAYERS):
                d = DILS[l]
                hb = hist[l]
                nzas, nzbs = {}, {}
                for pi, t in enumerate(group):
                    off = pi * TILE
                    # z' = (A-0.5)*B (= z/2; the 2x is folded into W_skip/W_out)
                    zp = zas[t][:]
                    nc.vector.scalar_tensor_tensor(
                        zp, zp, -0.5, zbs[t][:], ALU.add, ALU.mult)
                    # po first: it gates the residual add -> next row's taps
                    if l < LAYERS - 1:
                        d2 = DILS[l + 1]
                        po = ypool.tile([GATE_CH, TILE], f32, tag="y", name=f"po{t}")
                        nc.tensor.matmul(po[0:RES_CH, :], wout[:, l, :], zp,
                                         start=True, stop=True)
                    nc.tensor.matmul(sas[t][:], wskip[:, l, 0:128], zp,
                                     start=(l == 0), stop=(l == LAYERS - 1))
                    nc.tensor.matmul(sbs[t][:], wskip[:, l, 128:256], zp,
                                     start=(l == 0), stop=(l == LAYERS - 1))
                    if l < LAYERS - 1:
                        nc.vector.tensor_add(
                            hist[l + 1][:, 2 * d2 + off:2 * d2 + off + TILE],
                            hb[:, 2 * d + off:2 * d + off + TILE], po[0:RES_CH, :])
                        y = emit_y1(t, pi, l + 1, cts[t])
                        nzas[t], nzbs[t] = emit_sig(t, l + 1, y)

                zas, zbs = nzas, nzbs
                # shift history left by NS*TILE (keep last 2d cols); all
                # readers/sources of hist[l] are emitted by end of row l
                if tg < NT - NS:
                    nc.gpsimd.tensor_copy(hb[:, 0:2 * d],
                                          hb[:, NS * TILE:NS * TILE + 2 * d])

            pending_final = [(t, sas[t], sbs[t]) for t in group]

        for t, sa, sb in pending_final:
            emit_final(t, sa, sb)

    nc.compile()
    return nc


def _prep_params(inputs):
    """Host-side weight folding. Returns dict of DRAM arrays (shared by cores)."""
    import ml_dtypes
    bf16 = ml_dtypes.bfloat16
    f64 = np.float64
    W_first = inputs["W_first"].astype(f64)
    W_dil = inputs["W_dil"].astype(f64)
    b_dil = inputs["b_dil"].astype(f64)
    b_c = inputs["b_c"].astype(f64)
    W_c = inputs["W_c"].astype(f64)
    W_skip = inputs["W_skip"].astype(f64)
    b_skip = inputs["b_skip"].astype(f64)
    W_out = inputs["W_out"].astype(f64)
    b_out = inputs["b_out"].astype(f64)
    b_first = inputs["b_first"].astype(f64)
    W_last1 = inputs["W_last1"].astype(f64)
    b_last1 = inputs["b_last1"].astype(f64)
    W_last2 = inputs["W_last2"].astype(f64)
    b_last2 = inputs["b_last2"].astype(f64)

    bias_gate = np.zeros((LAYERS, GATE_CH), f64)
    r = np.zeros(RES_CH, f64)
    for l in range(LAYERS):
        bias_gate[l] = b_dil[l] + b_c[l] + W_dil[l].sum(axis=2) @ r
        bias_gate[l][0:RES_CH] *= 2.0          # sigma(2a) trick: a-rows x2
        r = S * (r + b_out[l])

    cl = np.array([S ** (LAYERS - 1)] + [S ** (LAYERS - l) for l in range(1, LAYERS)], dtype=f64)
    skips_init = (cl[:, None] * b_skip).sum(axis=0)  # [256]

    wdil = np.empty((LAYERS, RES_CH, 3 * GATE_CH), np.float32)
    wc = np.empty((LAYERS, CIN_CH, GATE_CH), bf16)
    wskip = np.empty((LAYERS, RES_CH, SKIP_CH), bf16)
    wout = np.empty((LAYERS, RES_CH, RES_CH), bf16)
    for l in range(LAYERS):
        for k in range(KW):
            blk = (W_dil[l, :, :, k] * (S ** l)).T.copy()
            blk[:, 0:RES_CH] *= 2.0            # a-rows x2
            wdil[l, :, k * GATE_CH:(k + 1) * GATE_CH] = blk
        wcb = W_c[l].T.copy()
        wcb[:, 0:RES_CH] *= 2.0                # a-rows x2
        wc[l] = wcb.astype(bf16)
        wskip[l] = (W_skip[l] * (2.0 * cl[l])).T.astype(bf16)      # z'=z/2 comp
        wout[l] = (W_out[l] * (2.0 * S ** (-l))).T.astype(bf16)    # z'=z/2 comp

    wfirst = np.empty((2, 128, 2 * RES_CH), bf16)
    for ci in range(2):
        for k in range(2):
            wfirst[ci, :, k * RES_CH:(k + 1) * RES_CH] = W_first[:, ci * 128:(ci + 1) * 128, k].T
    wlast1 = np.stack([W_last1[:, 0:128].T, W_last1[:, 128:256].T]).astype(bf16)
    wlast2 = np.stack([W_last2[:, 0:128].T, W_last2[:, 128:256].T]).astype(bf16)

    biases = np.zeros((128, 32), np.float32)
    biases[:, 0:LAYERS] = bias_gate.T          # cols 0..19: gate bias (2a 0:64 / b 64:128)
    biases[0:RES_CH, 20] = b_first
    biases[:, 21] = skips_init[0:128]
    biases[:, 22] = skips_init[128:256]
    biases[:, 23] = b_last1[0:128]
    biases[:, 24] = b_last1[128:256]
    biases[:, 25] = b_last2[0:128]
    biases[:, 26] = b_last2[128:256]

    return {
        "wdil": wdil, "wc": wc, "wskip": wskip, "wout": wout,
        "wfirst": wfirst, "wlast1": wlast1, "wlast2": wlast2, "biases": biases,
    }


def kernel(**inputs):
    from concourse.bass_utils import run_bass_kernel_spmd

    if "nc" not in _CACHE:
        _CACHE["nc"] = _build_nc()
    nc = _CACHE["nc"]

    import ml_dtypes
    bf16 = ml_dtypes.bfloat16
    params = _prep_params(inputs)
    x = np.asarray(inputs["x"], np.float32).astype(bf16)
    c = np.asarray(inputs["c"], np.float32).astype(bf16)

    in_maps = []
    for core in range(8):
        b, half = core // 2, core % 2
        if half == 0:
            xs = np.concatenate([np.zeros((OUT_CH, 1), bf16), x[b, :, 0:W]], axis=1)
            cs = c[b, :, 0:W]
        else:
            xs = x[b, :, H1_START - 1:T]
            cs = c[b, :, H1_START:T]
        m = dict(params)
        m["x"] = np.ascontiguousarray(xs)
        m["c"] = np.ascontiguousarray(cs)
        in_maps.append(m)

    res = run_bass_kernel_spmd(nc, in_maps, list(range(8)))
    _CACHE["last_results"] = res

    out = np.empty((B, OUT_CH, T), np.float32)
    for core in range(8):
        b, half = core // 2, core % 2
        o = res.results[core]["out"]
        if half == 0:
            out[b, :, 0:W] = o
        else:
            out[b, :, W:T] = o[:, W - (T - W):]
    return out
